# revision 1
# baseline (speedup 1.0000x reference)
"""AChebyKANLinear forward on 8 TRN2 NeuronCores (data-parallel over batch).

y = silu(x) @ W_base^T + einsum('bid,iod->bo', cos(n_d * arccos(tanh x)), gated_coeffs)

Strategy (v3, balanced hybrid bf16/fp8):
  cos(n*arccos(c)) = T_n(c), c = tanh(x). The device computes 12 cheap
  polynomial "columns" of c per feature whose exact Chebyshev expansion is
  tracked symbolically on the host; the host solves a small linear system to
  fold the change of basis into the matmul weights (plus the silu base path,
  13 matmul blocks total).

  Engine economics on TRN2 (per full-batch column op, 4096x256 per core):
    - PE bf16 block 6.8us, fp8 DoubleRow block ~3.9us (-2.97)
    - DVE: tensor_scalar 4x for bf16 (2.1), 2x_2p for fp8-out ts/copy (4.3),
      tensor_tensor 2x bf16 (4.3), scalar_tensor_tensor always 1x (8.5)
    - ACT ~1 elem/cycle (6.8-9)
  A block converted to fp8 saves 2.97us PE but costs >=4.27us DVE, so only
  blocks whose quantize is cheap are fp8, until DVE load meets PE load:
  fp8 = {T1,T2,T4,T6,T8 via ts/copy quantizes + T12-ish written fp8 directly
  by ACT Square}; bf16 = {T3/2, (T5+T3)/2, (T7+T1)/4, high-degree leaves,
  silu} read directly from the bf16 chain (no quantize op at all).
  Impurities (T3 in c5, T1 in c7, ...) are absorbed exactly by the solve;
  offsets (T0) go to the bias. stt ops are replaced by ts+tt via shared
  helpers e2=c2-1.5, e4=c4-1, e8=c8-1.

  silu stays bf16 always: its block dominates the fp8 noise budget (solo-fp8
  silu costs 3.7e-2 rel err vs the 2e-2 gate; this design sims at ~7e-3).

  Top-k routing over the 8 logits is computed on the host (8 numbers);
  the 4 selected high degrees are baked into the compiled graph.
"""

import numpy as np
import ml_dtypes
from contextlib import ExitStack

import concourse.bass as bass
import concourse.tile as tile
from concourse import bacc, mybir
from concourse.bass_utils import run_bass_kernel_spmd

BF16 = ml_dtypes.bfloat16

N_CORES = 8
BATCH, I_DIM, O_DIM = 32768, 256, 256
B_LOC = BATCH // N_CORES          # 4096
# graduated batch chunks: small first chunks shorten the pipeline fill before
# TensorE has all columns of chunk 0; steady state runs at 1024.
CHUNK_SIZES = [512, 512, 1024, 1024, 1024]
assert sum(CHUNK_SIZES) == B_LOC
BC_MAX = max(CHUNK_SIZES)
DEGREE = 16
BASE_DEGREES = 8
TOPK = 4

SQ2 = float(np.sqrt(2.0))

A = mybir.ActivationFunctionType
ALU = mybir.AluOpType
F32 = mybir.dt.float32
DBF16 = mybir.dt.bfloat16
DF8 = mybir.dt.float8e4
F8NP = mybir.dt.np(mybir.dt.float8e4)

W_SCALE = 4096.0

# engine assignment per op name ("scalar" = ACT, "vector" = DVE)
SCALAR_OPS = {"c1", "c2", "c4", "c6", "c8", "c10", "c12", "c16", "silu"}


# ---------------- symbolic Chebyshev algebra (host, exact) ----------------

def _chmul(a, b):
    out = np.zeros(40)
    for i in np.nonzero(a)[0]:
        for j in np.nonzero(b)[0]:
            p = a[i] * b[j] * 0.5
            out[i + j] += p
            out[abs(i - j)] += p
    return out


def _e(n):
    v = np.zeros(40)
    v[n] = 1.0
    return v


def _recipe(S):
    """Build the per-chunk op recipe.

    Returns (ops, vec, blocks) where ops is a list of
      ('act', name, src, func, scale, bias) or
      ('tt',  name, in0, in1, op) or
      ('ts',  name, in0, s1, op0, s2, op1) or   # (in op0 s1) op1 s2
      ('stt', name, in0, scalar, op0, in1, op1) or
      ('cp',  name, in0)
    vec maps name -> length-40 Chebyshev coefficient vector, and blocks is
    the ordered list of (name, is_fp8) matmul blocks (excluding silu).
    """
    ops = []
    vec = {}
    blocks = []

    def act(name, src, func, scale=1.0, bias=0.0):
        ops.append(("act", name, src, func, float(scale), float(bias)))
        if func == A.Square:
            aff = vec[src] * scale
            aff[0] += bias
            vec[name] = _chmul(aff, aff)
        elif func == A.Tanh:
            vec[name] = _e(1)
        else:
            vec[name] = None

    def tt(name, in0, in1, op):
        ops.append(("tt", name, in0, in1, op))
        if op == ALU.subtract:
            vec[name] = vec[in0] - vec[in1]
        elif op == ALU.add:
            vec[name] = vec[in0] + vec[in1]
        elif op == ALU.mult:
            vec[name] = _chmul(vec[in0], vec[in1])
        else:
            raise ValueError(op)

    def ts(name, in0, s1, op0, s2=None, op1=None):
        ops.append(("ts", name, in0, float(s1), op0,
                    None if s2 is None else float(s2), op1))
        a = vec[in0].copy()
        if op0 == ALU.add:
            a[0] += s1
        elif op0 == ALU.mult:
            a = a * s1
        else:
            raise ValueError(op0)
        if s2 is not None:
            if op1 == ALU.add:
                a[0] += s2
            elif op1 == ALU.mult:
                a = a * s2
            else:
                raise ValueError(op1)
        vec[name] = a

    def stt(name, in0, scalar, op0, in1, op1):
        ops.append(("stt", name, in0, float(scalar), op0, in1, op1))
        a = vec[in0].copy()
        if op0 == ALU.add:
            a[0] += scalar
        elif op0 == ALU.mult:
            a = a * scalar
        else:
            raise ValueError(op0)
        b = vec[in1]
        if op1 == ALU.mult:
            vec[name] = _chmul(a, b)
        elif op1 == ALU.subtract:
            vec[name] = a - b
        else:
            raise ValueError(op1)

    def cp(name, in0):
        ops.append(("cp", name, in0))
        vec[name] = vec[in0].copy()

    # chain; c1 first (unblocks everything), silu last (consumed last).
    act("c1", "x", A.Tanh)                      # T1        [bf16 block]
    act("c2", "c1", A.Square, SQ2)              # T2+1      [bf16]
    ts("f2", "c2", -1.0, ALU.add)               # T2        [fp8 block]
    ts("e2", "c2", -1.5, ALU.add)               # T2-1/2    [bf16]
    tt("c3", "e2", "c1", ALU.mult)              # T3/2      [bf16]
    cp("f3", "c3")                              # T3/2      [fp8 block]
    act("c4", "c2", A.Square, SQ2, -SQ2)        # T4+1      [bf16]
    ts("e4", "c4", -1.0, ALU.add)               # T4        [bf16 block]
    tt("c5", "e4", "c1", ALU.mult)              # (T5+T3)/2 [bf16]
    cp("f5", "c5")                              # (T5+T3)/2 [fp8 block]
    tt("c7", "e4", "c3", ALU.mult)              # (T7+T1)/4 [bf16]
    cp("f7", "c7")                              # (T7+T1)/4 [fp8 block]
    act("c6", "c3", A.Square, 2.0 * SQ2)        # T6+1      [bf16]
    ts("f6", "c6", -1.0, ALU.add)               # T6        [fp8 block]
    act("c8", "c4", A.Square, SQ2, -SQ2)        # T8+1      [bf16]
    ts("e8", "c8", -1.0, ALU.add)               # T8        [bf16]
    cp("f8", "e8")                              # T8        [fp8 block]
    blocks.extend([("c1", False), ("f2", True), ("f3", True), ("e4", False),
                   ("f5", True), ("f6", True), ("f7", True), ("f8", True)])

    if 10 in S:
        tt("d53", "c5", "c3", ALU.subtract)     # T5/2      [bf16]
    for n in sorted(S):
        if n == 9:
            tt("c9", "e8", "c1", ALU.mult)      # (T9+T7)/2  [bf16 block]
            blocks.append(("c9", False))
        elif n == 10:
            # (2*sqrt2*d53)^2 = 8*T5^2/4 = T10+1   [fp8 block, ACT direct]
            act("c10", "d53", A.Square, 2.0 * SQ2)
            blocks.append(("c10", True))
        elif n == 11:
            tt("c11", "e8", "c3", ALU.mult)     # (T11+T5)/4 [bf16 block]
            blocks.append(("c11", False))
        elif n == 12:
            # (sqrt2*c6 - sqrt2)^2 with c6=T6+1 -> 2*T6^2 = T12+1
            act("c12", "c6", A.Square, SQ2, -SQ2)
            blocks.append(("c12", True))        # T12+1 [fp8 block, ACT]
        elif n == 13:
            if 10 in S:
                tt("c13", "e8", "d53", ALU.mult)   # (T13+T3)/4 [bf16 block]
            else:
                # T8*(T5+T3)/2 = (T13+T3+T11+T5)/4; impurities all in span
                tt("c13", "e8", "c5", ALU.mult)
            blocks.append(("c13", False))
        elif n == 14:
            tt("c14", "e8", "c6", ALU.mult)     # (T14+T2)/2+T8 [bf16 block]
            blocks.append(("c14", False))
        elif n == 15:
            stt("t7p", "c7", 4.0, ALU.mult, "c1", ALU.subtract)  # T7
            tt("c15", "e8", "t7p", ALU.mult)    # (T15+T1)/2 [bf16 block]
            blocks.append(("c15", False))
        elif n == 16:
            act("c16", "e8", A.Square, SQ2)     # T16+1      [fp8 block, ACT]
            blocks.append(("c16", True))
        else:
            raise ValueError(n)
    act("silu", "x", A.Silu)
    return ops, vec, blocks


def _solve_basis(S, low_degrees):
    """Solve for X s.t. sum_col X[col,n]*vec[col] == e_n for each needed n."""
    ops, vec, blocks = _recipe(S)
    needed = sorted(set(int(n) for n in low_degrees) | set(S))
    Amat = np.zeros((40, 1 + len(blocks)))
    Amat[0, 0] = 1.0
    for j, (cn, _) in enumerate(blocks):
        Amat[:, 1 + j] = vec[cn]
    X = {}
    for n in needed:
        sol, res, rank, _ = np.linalg.lstsq(Amat, _e(n), rcond=None)
        err = np.abs(Amat @ sol - _e(n)).max()
        assert err < 1e-9, f"basis solve failed for degree {n}: {err}"
        X[n] = sol
    return ops, blocks, vec, X


# ---------------- device graph ----------------

def _build_nc(S, niter=1):
    ops, vec, cheb_blocks = _recipe(S)
    all_blocks = cheb_blocks + [("silu", False)]
    f8names = [cn for cn, f8 in all_blocks if f8]
    bfnames = [cn for cn, f8 in all_blocks if not f8]
    n8 = len(f8names)
    n16 = len(bfnames)
    f8slot = {cn: i for i, cn in enumerate(f8names)}
    bfslot = {cn: i for i, cn in enumerate(bfnames)}

    nc = bacc.Bacc("TRN2", target_bir_lowering=False, debug=False,
                   num_devices=N_CORES)
    bias_consts = sorted({op[5] for op in ops if op[0] == "act"} - {0.0})
    for v in bias_consts:
        t_c = nc.alloc_sbuf_tensor(f"const-f32-{v}", [128, 1], F32)
        nc.gpsimd.memset(t_c.ap(), v)
        nc.const_aps.aps[(F32, v)] = t_c.ap()
    if bias_consts:
        nc.all_engine_barrier()
    x_d = nc.dram_tensor("xt", [128, 2 * B_LOC], DBF16, kind="ExternalInput").ap()
    w_d = nc.dram_tensor("w", [128, n16 * 2 * O_DIM], DBF16,
                         kind="ExternalInput").ap()
    w8_d = nc.dram_tensor("w8", [128, n8 * 2 * O_DIM], DF8,
                          kind="ExternalInput").ap()
    b_d = nc.dram_tensor("bias", [O_DIM, 1], F32, kind="ExternalInput").ap()
    o_d = nc.dram_tensor("out", [O_DIM, B_LOC], F32, kind="ExternalOutput").ap()

    with tile.TileContext(nc) as tc, ExitStack() as ctx:
        cpool = ctx.enter_context(tc.tile_pool(name="const", bufs=1))
        xpool = ctx.enter_context(tc.tile_pool(name="x", bufs=3))
        tpool = ctx.enter_context(tc.tile_pool(name="tmp", bufs=2))
        fpool = ctx.enter_context(tc.tile_pool(name="f8", bufs=3))
        opool = ctx.enter_context(tc.tile_pool(name="o", bufs=8))
        pspool = ctx.enter_context(tc.tile_pool(name="ps", bufs=8, space="PSUM"))

        wt = cpool.tile([128, n16 * 2 * O_DIM], DBF16)
        nc.sync.dma_start(wt[:], w_d[:])
        w8t = cpool.tile([128, n8 * 2 * O_DIM], DF8)
        nc.sync.dma_start(w8t[:], w8_d[:])
        w8v = w8t[:].rearrange("p (b r o) -> p b r o", r=2, o=O_DIM)
        bt = []
        for m in range(2):
            b_tile = cpool.tile([128, 1], F32, tag=f"bias{m}")
            nc.sync.dma_start(b_tile[:], b_d[m * 128:(m + 1) * 128, :])
            bt.append(b_tile)

        chunks = []
        off = 0
        for bc in CHUNK_SIZES:
            chunks.append((off, bc))
            off += bc
        for it in range(niter):
            for ci, (off, bc) in enumerate(chunks):
                cc = f"{it}_{ci}"
                xt = xpool.tile([128, 2 * bc], DBF16, tag="xt", name=f"xt{cc}")
                nc.sync.dma_start(xt[:], x_d[:, 2 * off: 2 * (off + bc)])

                tiles = {"x": xt}
                for op in ops:
                    kind, name = op[0], op[1]
                    is_f8 = name in f8slot
                    dt_col = DF8 if is_f8 else DBF16
                    pool = fpool if is_f8 else tpool
                    t = pool.tile([128, 2 * bc], dt_col, tag=name,
                                  name=f"{name}_{cc}")
                    if kind == "act":
                        _, _, src, func, scale, bias_v = op
                        nc.scalar.activation(t[:], tiles[src][:], func,
                                             bias=bias_v, scale=scale)
                    elif kind == "tt":
                        _, _, in0, in1, alu = op
                        nc.vector.tensor_tensor(t[:], tiles[in0][:],
                                                tiles[in1][:], alu)
                    elif kind == "ts":
                        _, _, in0, s1, op0, s2, op1 = op
                        if s2 is None:
                            nc.vector.tensor_single_scalar(
                                t[:], tiles[in0][:], s1, op0)
                        else:
                            nc.vector.tensor_scalar(
                                t[:], tiles[in0][:], s1, s2, op0, op1)
                    elif kind == "stt":
                        _, _, in0, scalar, op0, in1, op1 = op
                        nc.vector.scalar_tensor_tensor(
                            t[:], tiles[in0][:], scalar, tiles[in1][:],
                            op0, op1)
                    else:  # cp
                        _, _, in0 = op
                        nc.vector.tensor_copy(t[:], tiles[in0][:])
                    tiles[name] = t

                nsubs = [(s, min(512, bc - s)) for s in range(0, bc, 512)]
                n_mm = n8 + 2 * n16
                for m in range(2):
                    for so, sn in nsubs:
                        ps = pspool.tile([128, sn], F32, tag="ps",
                                         name=f"ps{cc}_{m}_{so}")
                        mi = 0
                        for cn, is_f8 in all_blocks:
                            rt = tiles[cn]
                            if is_f8:
                                b8 = f8slot[cn]
                                rhs = rt[:].rearrange(
                                    "p (r n) -> p r n", r=2)[:, :, so:so + sn]
                                nc.tensor.matmul(
                                    ps[:],
                                    w8v[:, b8, :, m * 128:(m + 1) * 128],
                                    rhs,
                                    start=(mi == 0), stop=(mi == n_mm - 1),
                                    perf_mode=mybir.MatmulPerfMode.DoubleRow,
                                )
                                mi += 1
                            else:
                                j = bfslot[cn]
                                for kk in range(2):
                                    wcol = (2 * j + kk) * O_DIM
                                    nc.tensor.matmul(
                                        ps[:],
                                        wt[:, wcol + m * 128: wcol + (m + 1) * 128],
                                        rt[:, kk * bc + so: kk * bc + so + sn],
                                        start=(mi == 0), stop=(mi == n_mm - 1),
                                    )
                                    mi += 1
                        ot = opool.tile([128, sn], F32, tag="ot",
                                        name=f"ot{cc}_{m}_{so}")
                        nc.vector.tensor_scalar(ot[:], ps[:], 1.0 / W_SCALE,
                                                bt[m][:], ALU.mult, ALU.add)
                        nc.sync.dma_start(
                            o_d[m * 128:(m + 1) * 128, off + so: off + so + sn],
                            ot[:])

    nc.compile()
    return nc


_NC_CACHE = {}


def _get_nc(S, niter=1):
    key = (tuple(S), niter)
    if key not in _NC_CACHE:
        _NC_CACHE[key] = _build_nc(S, niter)
    return _NC_CACHE[key]


# ---------------- host wrapper ----------------

def _prepare(x, logits, cheby_coeffs, base_weight, gating_weights, arange):
    x = np.asarray(x, dtype=np.float32)
    logits = np.asarray(logits, dtype=np.float32)
    cheby_coeffs = np.asarray(cheby_coeffs, dtype=np.float32)
    base_weight = np.asarray(base_weight, dtype=np.float32)
    gating_weights = np.asarray(gating_weights, dtype=np.float32)
    arange = np.asarray(arange)

    # top-k routing (host; 8 numbers). Matches jax.lax.top_k ordering.
    order = np.argsort(-logits, kind="stable")[:TOPK]
    topk_vals = 1.0 / (1.0 + np.exp(-logits[order].astype(np.float64)))
    gate = gating_weights.astype(np.float64).copy()
    sel = order + BASE_DEGREES + 1
    gate[sel] = topk_vals
    S = sorted(int(v) for v in sel)

    low = sorted(int(v) for v in arange)   # normally [0..8]
    ops, cheb_blocks, vec, X = _solve_basis(S, low)
    all_blocks = cheb_blocks + [("silu", False)]
    f8names = [cn for cn, f8 in all_blocks if f8]
    bfnames = [cn for cn, f8 in all_blocks if not f8]

    G = {n: gate[n] * cheby_coeffs[:, :, n].astype(np.float64)
         for n in set(low) | set(S)}
    bias = np.zeros(O_DIM, dtype=np.float64)
    Wb = {"silu": base_weight.T.astype(np.float64)}
    for j, (cn, _) in enumerate(cheb_blocks):
        W = np.zeros((I_DIM, O_DIM), dtype=np.float64)
        for n, sol in X.items():
            coef = sol[1 + j]
            if coef != 0.0 and n in G:
                W += coef * G[n]
        Wb[cn] = W
    for n, sol in X.items():
        if sol[0] != 0.0 and n in G:
            bias += sol[0] * G[n].sum(axis=0)

    W8sb = np.empty((128, 2 * len(f8names), O_DIM), dtype=F8NP)
    for b8, cn in enumerate(f8names):
        Wq = (Wb[cn] * W_SCALE).astype(np.float32).astype(F8NP)
        W8sb[:, 2 * b8 + 0, :] = Wq[0:128, :]
        W8sb[:, 2 * b8 + 1, :] = Wq[128:256, :]
    w8_np = W8sb.reshape(128, 2 * len(f8names) * O_DIM)

    Wsb = np.empty((128, 2 * len(bfnames), O_DIM), dtype=np.float32)
    for j, cn in enumerate(bfnames):
        Wf = (Wb[cn] * W_SCALE).astype(np.float32)
        Wsb[:, 2 * j + 0, :] = Wf[0:128, :]
        Wsb[:, 2 * j + 1, :] = Wf[128:256, :]
    w_np = Wsb.reshape(128, 2 * len(bfnames) * O_DIM).astype(BF16)

    bias_np = bias.astype(np.float32).reshape(O_DIM, 1)
    return S, w_np, w8_np, bias_np


def _make_xt(xl):
    """xt[p, 2*off + h*bc + bb] = xl[off+bb, 128*h+p] for each chunk (off, bc)."""
    out = np.empty((128, 2 * B_LOC), dtype=BF16)
    off = 0
    for bc in CHUNK_SIZES:
        blk = xl[off:off + bc, :].reshape(bc, 2, 128).transpose(2, 1, 0)
        out[:, 2 * off: 2 * (off + bc)] = blk.reshape(128, 2 * bc)
        off += bc
    return out


def _make_in_maps(x, w_np, w8_np, bias_np):
    in_maps = []
    for c in range(N_CORES):
        m = {"xt": _make_xt(x[c * B_LOC:(c + 1) * B_LOC, :]),
             "w": w_np, "bias": bias_np, "w8": w8_np}
        in_maps.append(m)
    return in_maps


def kernel(x, t, logits, cheby_coeffs, base_weight, gating_weights, arange):
    x = np.asarray(x, dtype=np.float32)
    S, w_np, w8_np, bias_np = _prepare(x, logits, cheby_coeffs, base_weight,
                                       gating_weights, arange)
    nc = _get_nc(S)
    in_maps = _make_in_maps(x, w_np, w8_np, bias_np)
    res = run_bass_kernel_spmd(nc, in_maps, core_ids=list(range(N_CORES)))
    y = np.empty((BATCH, O_DIM), dtype=np.float32)
    for c in range(N_CORES):
        y[c * B_LOC:(c + 1) * B_LOC, :] = res.results[c]["out"].T
    return y

